# revision 7
# baseline (speedup 1.0000x reference)
"""Trainium2 Bass kernel for nn_Conv_34187939676169.

The model applies 8 conv2d(1->1, 3x3, pad 1) layers to N=4M independent 3x3
patches. On a 3x3 grid each conv layer is a linear map on the flattened
9-vector, so the whole stack is a single affine map y = M @ x + c with
M = A_7 @ ... @ A_0 (9x9) and c the accumulated biases. M and c are computed
on the host in float64 from the (tiny) weight/bias inputs; the device kernel
streams the 4M x 9 data through the TensorEngine:

  per [128, 126] tile (128 partitions x 14 patches x 9 components):
    PE transpose -> [126, 128] PSUM  (data gets the 9-dim onto partitions)
    ACT copy PSUM -> SBUF (bf16)
    PE matmul(lhsT = transposed data [126,128], rhs = kron(I_14, M^T) [126,126])
       -> natural-layout output [128, 126] in PSUM (fp32)
    DVE tensor_add(psum, bias_tile) -> SBUF fp32
  DMA: input is cast fp32->bf16 in-flight (SWDGE); output written fp32.

Sharding: pure data parallel over 8 cores. Each core gets an overlapping
slice of 501760 rows (= 280 uniform tiles), so a single SPMD program with no
ragged tail covers all 4,000,000 rows; overlapped rows are computed twice and
overwritten with identical values at gather time.
"""

import os
import sys

sys.path.insert(0, "/opt/trn_rl_repo")

import numpy as np
import ml_dtypes

import concourse.bass as bass
import concourse.bacc as bacc
import concourse.tile as tile
from concourse import mybir
from concourse.bass_utils import run_bass_kernel_spmd

P = 128              # SBUF partitions
G = 14               # patches per partition per tile
TILE_COLS = G * 9    # 126
ROWS_PER_TILE = P * G  # 1792
QU = 4               # tiles per PSUM batch ("quad")

N_CORES = 8
N_TOTAL = 4_000_000

# Full-size config: 280 tiles/core; small lead-in chunks get the output
# stream started ASAP (so input/output DMA share bandwidth ~50/50 through
# the steady state), small lead-out chunks shorten the store tail. Big
# chunks are loaded in one DMA but laid out as two independent 16-tile
# halves (s=2 rearrange) so each half can be stored as soon as its
# compute finishes.
CHUNK_TILES = [4, 8, 16] + [32] * 7 + [16, 8, 4]
TILES_PC = sum(CHUNK_TILES)                    # 280
ROWS_PC = TILES_PC * ROWS_PER_TILE             # 501760

BF16 = mybir.dt.bfloat16
F32 = mybir.dt.float32


def _conv_matrix(w: np.ndarray) -> np.ndarray:
    """9x9 matrix of conv2d(1->1, 3x3, pad 1) on a flattened 3x3 grid.

    Cross-correlation (torch/jax convention):
      out[r,s] = sum_{a,b} w[a,b] * in[r+a-1, s+b-1], zero padded.
    """
    A = np.zeros((9, 9), dtype=np.float64)
    for r in range(3):
        for s in range(3):
            for a in range(3):
                for b in range(3):
                    rr, ss = r + a - 1, s + b - 1
                    if 0 <= rr < 3 and 0 <= ss < 3:
                        A[r * 3 + s, rr * 3 + ss] += w[a, b]
    return A


def _affine(weights: np.ndarray, biases: np.ndarray):
    """Compose the depth-D stack into y = M @ x + c (float64)."""
    M = np.eye(9, dtype=np.float64)
    c = np.zeros(9, dtype=np.float64)
    for d in range(weights.shape[0]):
        A = _conv_matrix(np.asarray(weights[d], dtype=np.float64).reshape(3, 3))
        M = A @ M
        c = A @ c + float(biases[d])
    return M, c


def _build_nc(chunk_tiles, cast_in_dma: bool = True):
    """chunk_tiles: list of per-chunk tile counts (uneven allowed).

    A small first chunk shortens the pipeline-fill stall (first transposes
    wait only for a small DMA); a smaller last chunk shortens the store
    tail after the final compute."""
    total_tiles = sum(chunk_tiles)
    rows = total_tiles * ROWS_PER_TILE
    max_chunk = max(chunk_tiles)

    tdt = BF16 if cast_in_dma else F32  # dtype of the pre-transpose data path

    nc = bacc.Bacc("TRN2", target_bir_lowering=False)
    x = nc.dram_tensor("x", [rows, 9], F32, kind="ExternalInput")
    y = nc.dram_tensor("y", [rows, 9], F32, kind="ExternalOutput")
    ident = nc.dram_tensor("ident", [P, P], tdt, kind="ExternalInput")
    # rows 0..125: kron(I_14, M^T); rows 126/127: hi/lo bf16 split of bias c
    rmat = nc.dram_tensor("rmat", [P, TILE_COLS], BF16, kind="ExternalInput")

    with tile.TileContext(nc) as tc:
        with (
            tc.tile_pool(name="consts", bufs=1) as cpool,
            tc.tile_pool(name="inp", bufs=4) as inpool,
            tc.tile_pool(name="outp", bufs=5) as outpool,
            tc.tile_pool(name="xts", bufs=4) as xtpool,
            tc.tile_pool(name="pst", bufs=3, space="PSUM") as pst,
            tc.tile_pool(name="psy", bufs=5, space="PSUM") as psy,
        ):
            ident_s = cpool.tile([P, P], tdt)
            nc.sync.dma_start(ident_s[:], ident[:])
            r_s = cpool.tile([P, TILE_COLS], BF16)
            nc.sync.dma_start(r_s[:], rmat[:])

            # Issue ALL input-chunk loads up front on the gpsimd (SWDGE)
            # queue: the first load is then the engine's first instruction
            # (nothing serializes ahead of it), and later loads self-pace
            # on in_t slot availability (inpool bufs). Chunks > 16 tiles
            # are laid out as two independent halves (s=2) so each half
            # can be stored independently.
            in_tiles = []
            tile_base = 0
            for ch, ctiles in enumerate(chunk_tiles):
                rows_per_chunk = ctiles * ROWS_PER_TILE
                row0 = tile_base * ROWS_PER_TILE
                tile_base += ctiles
                nsub = 2 if ctiles > 16 else 1
                sub = ctiles // nsub
                in_t = inpool.tile(
                    [P, max_chunk * TILE_COLS], tdt, tag="in_t", name="in_t"
                )[:, : ctiles * TILE_COLS]
                for s in range(nsub):
                    xin = x[
                        row0 + s * sub * ROWS_PER_TILE :
                        row0 + (s + 1) * sub * ROWS_PER_TILE,
                        :,
                    ].rearrange("(p r) c -> p (r c)", p=P)
                    dst = in_t[:, s * sub * TILE_COLS : (s + 1) * sub * TILE_COLS]
                    if cast_in_dma:
                        # SWDGE DMA converts fp32 -> bf16 in flight
                        nc.gpsimd.dma_start(dst, xin)
                    else:
                        nc.sync.dma_start(dst, xin)
                in_tiles.append(in_t)

            # Persistent lhsT tiles: rows 0..125 receive transposed data each
            # quad; rows 126/127 stay 1.0 forever so the matmul contraction
            # picks up the bias rows of rmat. Memset on the (otherwise idle
            # at startup) vector engine so nothing delays the first load.
            xt_tiles = [
                xtpool.tile([P, QU * P], BF16, tag=f"xt{i}", name=f"xt{i}")
                for i in range(4)
            ]
            for t_ in xt_tiles:
                # partition slices must start at a multiple of 32; rows
                # 96..125 get overwritten with data by every quad's copy,
                # rows 126/127 stay 1.0 forever.
                nc.vector.memset(t_[96:P, :], 1.0)

            qglob = 0
            tile_base = 0
            for ch, ctiles in enumerate(chunk_tiles):
                rows_per_chunk = ctiles * ROWS_PER_TILE
                cols_per_chunk = ctiles * TILE_COLS
                row0 = tile_base * ROWS_PER_TILE
                tile_base += ctiles
                groups = []
                g0 = 0
                while g0 < ctiles:
                    g = min(QU, ctiles - g0)
                    groups.append((g0, g))
                    g0 += g
                in_t = in_tiles[ch]

                out_t = outpool.tile(
                    [P, max_chunk * TILE_COLS], F32, tag="out_t", name="out_t"
                )[:, :cols_per_chunk]
                # Big chunks (s=2 layout halves) get two stores so output
                # DMA starts before the whole chunk's compute finishes.
                nsub = 2 if ctiles > 16 else 1
                sub = ctiles // nsub
                store_groups = [(s * sub, sub) for s in range(nsub)]
                for q, (tbase, gsz) in enumerate(groups):
                    xt_ps = pst.tile([TILE_COLS, QU * P], tdt)
                    for s_ in range(gsz):
                        t = tbase + s_
                        nc.tensor.transpose(
                            xt_ps[:, s_ * P : (s_ + 1) * P],
                            in_t[:, t * TILE_COLS : (t + 1) * TILE_COLS],
                            ident_s[:],
                        )
                    xt_sb = xt_tiles[qglob % 4]
                    qglob += 1
                    nc.vector.tensor_copy(
                        xt_sb[:TILE_COLS, : gsz * P], xt_ps[:, : gsz * P]
                    )

                    y_ps = psy.tile([P, QU * TILE_COLS], F32)
                    for s_ in range(gsz):
                        nc.tensor.matmul(
                            y_ps[:, s_ * TILE_COLS : (s_ + 1) * TILE_COLS],
                            xt_sb[:, s_ * P : (s_ + 1) * P],
                            r_s[:],
                            start=True,
                            stop=True,
                        )
                    nc.scalar.copy(
                        out_t[
                            :,
                            tbase * TILE_COLS : (tbase + gsz) * TILE_COLS,
                        ],
                        y_ps[:, : gsz * TILE_COLS],
                    )

                for sbase, scnt in store_groups:
                    yout = y[
                        row0 + sbase * ROWS_PER_TILE :
                        row0 + (sbase + scnt) * ROWS_PER_TILE,
                        :,
                    ].rearrange("(p r) c -> p (r c)", p=P)
                    nc.sync.dma_start(
                        yout,
                        out_t[:, sbase * TILE_COLS : (sbase + scnt) * TILE_COLS],
                    )
    nc.compile()
    return nc


def _make_consts(M: np.ndarray, c: np.ndarray, cast_in_dma: bool = True):
    tdt_np = ml_dtypes.bfloat16 if cast_in_dma else np.float32
    ident = np.eye(P, dtype=tdt_np)
    rmat = np.zeros((P, TILE_COLS), dtype=ml_dtypes.bfloat16)
    # R[9k+j, 9k+i] = M[i, j]  ->  block-diagonal of M^T
    rmat[:TILE_COLS, :] = np.kron(np.eye(G, dtype=np.float64), M.T).astype(
        ml_dtypes.bfloat16
    )
    # bias via the two all-ones lhsT rows: c = c_hi + c_lo (bf16 hi/lo split)
    c_hi = c.astype(ml_dtypes.bfloat16)
    c_lo = (c - c_hi.astype(np.float64)).astype(ml_dtypes.bfloat16)
    rmat[TILE_COLS, :] = np.tile(c_hi, G)
    rmat[TILE_COLS + 1, :] = np.tile(c_lo, G)
    return {"ident": ident, "rmat": rmat}


_NC_CACHE: dict = {}


def _get_nc(key, builder):
    if key not in _NC_CACHE:
        _NC_CACHE[key] = builder()
    return _NC_CACHE[key]


def kernel(input: np.ndarray, weights: np.ndarray, biases: np.ndarray) -> np.ndarray:
    x = np.ascontiguousarray(np.asarray(input, dtype=np.float32))
    n = x.shape[0]
    assert x.shape == (N_TOTAL, 9), f"unexpected input shape {x.shape}"

    M, c = _affine(np.asarray(weights), np.asarray(biases))

    cast_in_dma = os.environ.get("NNCONV_CAST_DMA", "1") == "1"
    trace = os.environ.get("NNCONV_TRACE", "0") == "1"

    nc = _get_nc(
        ("full", tuple(CHUNK_TILES), cast_in_dma),
        lambda: _build_nc(CHUNK_TILES, cast_in_dma),
    )
    consts = _make_consts(M, c, cast_in_dma)

    # Overlapping shards: core i covers rows [s_i, s_i + ROWS_PC)
    starts = [(n - ROWS_PC) * i // (N_CORES - 1) for i in range(N_CORES)]
    in_maps = []
    for s in starts:
        in_maps.append(
            {
                "x": np.ascontiguousarray(x[s : s + ROWS_PC]),
                **consts,
            }
        )

    res = run_bass_kernel_spmd(
        nc, in_maps, core_ids=list(range(N_CORES)), trace=trace
    )
    global _LAST_RESULTS
    _LAST_RESULTS = res
    if trace and res.exec_time_ns is not None:
        print(f"HW exec time: {res.exec_time_ns} ns")
        if res.instructions_and_trace is not None:
            print(f"trace: {res.instructions_and_trace[1]}")

    out = np.empty((n, 9), dtype=np.float32)
    for s, r in zip(starts, res.results):
        out[s : s + ROWS_PC] = r["y"]
    return out



# revision 24
# speedup vs baseline: 1.0846x; 1.0846x over previous
"""Trainium2 Bass kernel for nn_Conv_34187939676169.

The model applies 8 conv2d(1->1, 3x3, pad 1) layers to N=4M independent 3x3
patches. On a 3x3 grid each conv layer is a linear map on the flattened
9-vector, so the whole stack is a single affine map y = M @ x + c with
M = A_7 @ ... @ A_0 (9x9) and c the accumulated biases. M and c are computed
on the host in float64 from the (tiny) weight/bias inputs; the device kernel
streams the 4M x 9 data through the TensorEngine:

  per [128, 126] tile (128 partitions x 14 patches x 9 components):
    PE transpose -> [126(+2 junk), 128] PSUM (9-dim onto partitions; the
       stationary operand is padded to 128 cols to trigger fast-weight-load)
    DVE copy PSUM -> SBUF (bf16)
    PE matmul(lhsT = transposed data [128,128] incl. two all-ones rows,
              rhs = kron(I_14, M^T) + bias rows [128,126])
       -> natural-layout output [128, 126] in PSUM (fp32)
    ACT copy PSUM -> SBUF fp32 (optionally DVE-assisted in the drain tail)
  DMA: input is cast fp32->bf16 in-flight (SWDGE); output written fp32.

The kernel is wire-limited: the 16 SDMA engines/core sustain ~27 GB/s each
(~432 GB/s aggregate), and with 18.1 MB read + 18.1 MB written per core the
steady-state stream floor is ~84 us. The schedule therefore optimizes the
edges: all input loads are queued on the SWDGE engine up front (first load
issues immediately at kernel entry; the ones-row memsets run on the
otherwise-idle vector engine), loads use uniform 28-tile ranges (fat DMA
descriptors ~= full packets), and stores taper 4/8/16 ... 16/8/4 over one
global rearranged view so the output stream starts early and the final
compute->store->receipt chain is short.

Sharding: pure data parallel over 8 cores. Each core gets an overlapping
slice of 501760 rows (= 280 uniform tiles), so a single SPMD program with no
ragged tail covers all 4,000,000 rows; overlapped rows are computed twice and
overwritten with identical values at gather time.
"""

import os
import sys

sys.path.insert(0, "/opt/trn_rl_repo")

import numpy as np
import ml_dtypes

import concourse.bass as bass
import concourse.bacc as bacc
import concourse.tile as tile
from concourse import mybir
from concourse.bass_utils import run_bass_kernel_spmd

P = 128              # SBUF partitions
G = 14               # patches per partition per tile
TILE_COLS = G * 9    # 126
ROWS_PER_TILE = P * G  # 1792
QU = 4               # tiles per PSUM batch ("quad")

N_CORES = 8
N_TOTAL = 4_000_000

BF16 = mybir.dt.bfloat16
F32 = mybir.dt.float32

# Tunable schedule configuration (overridable for A/B benching).
# Defaults = best measured config ("g1p"): global-view pipeline, 28-tile
# loads with small lead/tail, store taper 4/8/16 ... 16/8/4, all loads
# queued upfront on SWDGE, ones-memsets on DVE, FWL-padded transposes.
DEFAULT_CFG = dict(
    chunk_tiles=(8, 28, 28, 28, 28, 28, 28, 28, 28, 24, 16, 8),
    inbufs=3,
    outbufs=4,
    pst_bufs=4,
    psy_bufs=4,
    loads_upfront=True,    # emit all input dma_starts before compute loop
    memset_vector=True,    # ones-rows memset on DVE instead of GpSimd
    store_split=0,         # split stores of chunks larger than this (0=off)
    cast_in_dma=True,
    # When load_tiles/store_tiles are set, the "global view" path is used:
    # one rearranged view of the whole shard; loads and stores are
    # independent column-range slices of it (granularity decoupled).
    load_tiles=(12,) + (28,) * 9 + (16,),
    store_tiles=(4, 8, 16) + (28,) * 8 + (16, 8, 4),
    # Pad transpose stationary operands to 128 columns so LDWEIGHTS gets
    # fast-weight-load (needs 2 slack columns in the in_t allocation).
    pad_transpose=True,
    # Split each quad's PSUM->SBUF y copy: DVE takes the first split_y
    # tiles, ACT the rest (0 = ACT does all 4). Balances the copy engines
    # so the post-input drain can feed the store DMAs at full rate.
    split_y=0,
    # Only apply split_y to quads at tile index >= split_y_from (the
    # ACT-paced drain region after input DMA ends); 0 = apply everywhere.
    split_y_from=0,
)

TILES_PC = 280
ROWS_PC = TILES_PC * ROWS_PER_TILE             # 501760


def _conv_matrix(w: np.ndarray) -> np.ndarray:
    """9x9 matrix of conv2d(1->1, 3x3, pad 1) on a flattened 3x3 grid.

    Cross-correlation (torch/jax convention):
      out[r,s] = sum_{a,b} w[a,b] * in[r+a-1, s+b-1], zero padded.
    """
    A = np.zeros((9, 9), dtype=np.float64)
    for r in range(3):
        for s in range(3):
            for a in range(3):
                for b in range(3):
                    rr, ss = r + a - 1, s + b - 1
                    if 0 <= rr < 3 and 0 <= ss < 3:
                        A[r * 3 + s, rr * 3 + ss] += w[a, b]
    return A


def _affine(weights: np.ndarray, biases: np.ndarray):
    """Compose the depth-D stack into y = M @ x + c (float64)."""
    M = np.eye(9, dtype=np.float64)
    c = np.zeros(9, dtype=np.float64)
    for d in range(weights.shape[0]):
        A = _conv_matrix(np.asarray(weights[d], dtype=np.float64).reshape(3, 3))
        M = A @ M
        c = A @ c + float(biases[d])
    return M, c


def _build_nc_global(cfg):
    """Global-view pipeline: load/store granularities are decoupled column
    ranges of one (p r)-rearranged view of the whole per-core shard."""
    load_tiles = list(cfg["load_tiles"])
    store_tiles = list(cfg["store_tiles"])
    assert sum(load_tiles) == TILES_PC and sum(store_tiles) == TILES_PC
    assert all(t % QU == 0 for t in load_tiles + store_tiles)
    cast_in_dma = cfg["cast_in_dma"]
    pad_t = cfg["pad_transpose"]
    rows = TILES_PC * ROWS_PER_TILE
    max_load = max(load_tiles)
    max_store = max(store_tiles)

    tdt = BF16 if cast_in_dma else F32

    nc = bacc.Bacc("TRN2", target_bir_lowering=False)
    x = nc.dram_tensor("x", [rows, 9], F32, kind="ExternalInput")
    y = nc.dram_tensor("y", [rows, 9], F32, kind="ExternalOutput")
    ident = nc.dram_tensor("ident", [P, P], tdt, kind="ExternalInput")
    rmat = nc.dram_tensor("rmat", [P, TILE_COLS], BF16, kind="ExternalInput")

    xv = x.rearrange("(p r) c -> p (r c)", p=P)   # [128, 280*126]
    yv = y.rearrange("(p r) c -> p (r c)", p=P)

    with tile.TileContext(nc) as tc:
        with (
            tc.tile_pool(name="consts", bufs=1) as cpool,
            tc.tile_pool(name="inp", bufs=cfg["inbufs"]) as inpool,
            tc.tile_pool(name="outp", bufs=cfg["outbufs"]) as outpool,
            tc.tile_pool(name="xts", bufs=4) as xtpool,
            tc.tile_pool(name="pst", bufs=cfg["pst_bufs"], space="PSUM") as pst,
            tc.tile_pool(name="psy", bufs=cfg["psy_bufs"], space="PSUM") as psy,
        ):
            ident_s = cpool.tile([P, P], tdt)
            nc.sync.dma_start(ident_s[:], ident[:])
            r_s = cpool.tile([P, TILE_COLS], BF16)
            nc.sync.dma_start(r_s[:], rmat[:])

            # All input loads queued on the SWDGE engine up front;
            # they self-pace on in_t slot availability. With pad_transpose
            # the alloc has 2 slack columns so the last tile of each load
            # can over-read a 128-wide window (junk lands in transpose
            # rows 126/127, discarded by the PSUM->SBUF copy).
            in_tiles = []     # (tile_base, ntiles, sbuf tile incl. slack)
            tb = 0
            for ltiles in load_tiles:
                in_full = inpool.tile(
                    [P, max_load * TILE_COLS + (2 if pad_t else 0)],
                    tdt, tag="in_t", name="in_t",
                )
                src = xv[:, tb * TILE_COLS : (tb + ltiles) * TILE_COLS]
                dst = in_full[:, : ltiles * TILE_COLS]
                if cast_in_dma:
                    nc.gpsimd.dma_start(dst, src)
                else:
                    nc.sync.dma_start(dst, src)
                in_tiles.append((tb, ltiles, in_full))
                tb += ltiles

            xt_tiles = [
                xtpool.tile([P, QU * P], BF16, tag=f"xt{i}", name=f"xt{i}")
                for i in range(4)
            ]
            for t_ in xt_tiles:
                if cfg["memset_vector"]:
                    nc.vector.memset(t_[96:P, :], 1.0)
                else:
                    nc.gpsimd.memset(t_[96:P, :], 1.0)

            def in_slice(t):
                # load ranges and quads are QU-aligned, so a quad never
                # straddles a load range; lookup per tile is still exact.
                for tb_, lt_, tile_ in in_tiles:
                    if tb_ <= t < tb_ + lt_:
                        c0 = (t - tb_) * TILE_COLS
                        w = P if pad_t else TILE_COLS
                        return tile_[:, c0 : c0 + w]
                raise AssertionError(t)

            qglob = 0
            sb = 0
            for stiles in store_tiles:
                out_t = outpool.tile(
                    [P, max_store * TILE_COLS], F32, tag="out_t", name="out_t"
                )[:, : stiles * TILE_COLS]
                for tbase in range(sb, sb + stiles, QU):
                    gsz = min(QU, sb + stiles - tbase)
                    xt_ps = pst.tile([P if pad_t else TILE_COLS, QU * P], tdt)
                    for s_ in range(gsz):
                        nc.tensor.transpose(
                            xt_ps[:, s_ * P : (s_ + 1) * P],
                            in_slice(tbase + s_),
                            ident_s[:],
                        )
                    xt_sb = xt_tiles[qglob % 4]
                    qglob += 1
                    nc.vector.tensor_copy(
                        xt_sb[:TILE_COLS, : gsz * P],
                        xt_ps[:TILE_COLS, : gsz * P],
                    )
                    y_ps = psy.tile([P, QU * TILE_COLS], F32)
                    for s_ in range(gsz):
                        nc.tensor.matmul(
                            y_ps[:, s_ * TILE_COLS : (s_ + 1) * TILE_COLS],
                            xt_sb[:, s_ * P : (s_ + 1) * P],
                            r_s[:],
                            start=True,
                            stop=True,
                        )
                    o0 = (tbase - sb) * TILE_COLS
                    nsp = min(cfg["split_y"], gsz - 1)
                    if tbase < cfg["split_y_from"]:
                        nsp = 0
                    if nsp > 0:
                        nc.vector.tensor_copy(
                            out_t[:, o0 : o0 + nsp * TILE_COLS],
                            y_ps[:, : nsp * TILE_COLS],
                        )
                    nc.scalar.copy(
                        out_t[:, o0 + nsp * TILE_COLS : o0 + gsz * TILE_COLS],
                        y_ps[:, nsp * TILE_COLS : gsz * TILE_COLS],
                    )
                nc.sync.dma_start(
                    yv[:, sb * TILE_COLS : (sb + stiles) * TILE_COLS], out_t[:]
                )
                sb += stiles
    nc.compile()
    return nc


def _build_nc(cfg):
    if cfg.get("load_tiles"):
        return _build_nc_global(cfg)
    chunk_tiles = list(cfg["chunk_tiles"])
    cast_in_dma = cfg["cast_in_dma"]
    assert sum(chunk_tiles) == TILES_PC, sum(chunk_tiles)
    max_chunk = max(chunk_tiles)
    rows = TILES_PC * ROWS_PER_TILE

    tdt = BF16 if cast_in_dma else F32  # dtype of the pre-transpose data path

    nc = bacc.Bacc("TRN2", target_bir_lowering=False)
    x = nc.dram_tensor("x", [rows, 9], F32, kind="ExternalInput")
    y = nc.dram_tensor("y", [rows, 9], F32, kind="ExternalOutput")
    ident = nc.dram_tensor("ident", [P, P], tdt, kind="ExternalInput")
    # rows 0..125: kron(I_14, M^T); rows 126/127: hi/lo bf16 split of bias c
    rmat = nc.dram_tensor("rmat", [P, TILE_COLS], BF16, kind="ExternalInput")

    def store_groups_of(ctiles):
        if cfg["store_split"] and ctiles > cfg["store_split"]:
            sub = ctiles // 2
            return [(0, sub), (sub, ctiles - sub)]
        return [(0, ctiles)]

    with tile.TileContext(nc) as tc:
        with (
            tc.tile_pool(name="consts", bufs=1) as cpool,
            tc.tile_pool(name="inp", bufs=cfg["inbufs"]) as inpool,
            tc.tile_pool(name="outp", bufs=cfg["outbufs"]) as outpool,
            tc.tile_pool(name="xts", bufs=4) as xtpool,
            tc.tile_pool(name="pst", bufs=cfg["pst_bufs"], space="PSUM") as pst,
            tc.tile_pool(name="psy", bufs=cfg["psy_bufs"], space="PSUM") as psy,
        ):
            ident_s = cpool.tile([P, P], tdt)
            nc.sync.dma_start(ident_s[:], ident[:])
            r_s = cpool.tile([P, TILE_COLS], BF16)
            nc.sync.dma_start(r_s[:], rmat[:])

            chunk_row0 = []
            base = 0
            for ctiles in chunk_tiles:
                chunk_row0.append(base * ROWS_PER_TILE)
                base += ctiles

            def load_chunk(ch):
                ctiles = chunk_tiles[ch]
                row0 = chunk_row0[ch]
                in_t = inpool.tile(
                    [P, max_chunk * TILE_COLS], tdt, tag="in_t", name="in_t"
                )[:, : ctiles * TILE_COLS]
                for sbase, scnt in store_groups_of(ctiles):
                    xin = x[
                        row0 + sbase * ROWS_PER_TILE :
                        row0 + (sbase + scnt) * ROWS_PER_TILE,
                        :,
                    ].rearrange("(p r) c -> p (r c)", p=P)
                    dst = in_t[:, sbase * TILE_COLS : (sbase + scnt) * TILE_COLS]
                    if cast_in_dma:
                        # SWDGE DMA converts fp32 -> bf16 in flight
                        nc.gpsimd.dma_start(dst, xin)
                    else:
                        nc.sync.dma_start(dst, xin)
                return in_t

            in_tiles = {}
            if cfg["loads_upfront"]:
                for ch in range(len(chunk_tiles)):
                    in_tiles[ch] = load_chunk(ch)

            # Persistent lhsT tiles: rows 0..125 receive transposed data each
            # quad; rows 126/127 stay 1.0 forever so the matmul contraction
            # picks up the bias rows of rmat.
            xt_tiles = [
                xtpool.tile([P, QU * P], BF16, tag=f"xt{i}", name=f"xt{i}")
                for i in range(4)
            ]
            for t_ in xt_tiles:
                # partition slices must start at a multiple of 32; rows
                # 96..125 get overwritten with data by every quad's copy,
                # rows 126/127 stay 1.0 forever.
                if cfg["memset_vector"]:
                    nc.vector.memset(t_[96:P, :], 1.0)
                else:
                    nc.gpsimd.memset(t_[96:P, :], 1.0)

            qglob = 0
            for ch, ctiles in enumerate(chunk_tiles):
                cols_per_chunk = ctiles * TILE_COLS
                row0 = chunk_row0[ch]
                groups = []
                g0 = 0
                while g0 < ctiles:
                    g = min(QU, ctiles - g0)
                    groups.append((g0, g))
                    g0 += g
                in_t = in_tiles[ch] if cfg["loads_upfront"] else load_chunk(ch)

                out_t = outpool.tile(
                    [P, max_chunk * TILE_COLS], F32, tag="out_t", name="out_t"
                )[:, :cols_per_chunk]
                for _, (tbase, gsz) in enumerate(groups):
                    xt_ps = pst.tile([TILE_COLS, QU * P], tdt)
                    for s_ in range(gsz):
                        t = tbase + s_
                        nc.tensor.transpose(
                            xt_ps[:, s_ * P : (s_ + 1) * P],
                            in_t[:, t * TILE_COLS : (t + 1) * TILE_COLS],
                            ident_s[:],
                        )
                    xt_sb = xt_tiles[qglob % 4]
                    qglob += 1
                    nc.vector.tensor_copy(
                        xt_sb[:TILE_COLS, : gsz * P], xt_ps[:, : gsz * P]
                    )

                    y_ps = psy.tile([P, QU * TILE_COLS], F32)
                    for s_ in range(gsz):
                        nc.tensor.matmul(
                            y_ps[:, s_ * TILE_COLS : (s_ + 1) * TILE_COLS],
                            xt_sb[:, s_ * P : (s_ + 1) * P],
                            r_s[:],
                            start=True,
                            stop=True,
                        )
                    nc.scalar.copy(
                        out_t[
                            :,
                            tbase * TILE_COLS : (tbase + gsz) * TILE_COLS,
                        ],
                        y_ps[:, : gsz * TILE_COLS],
                    )

                for sbase, scnt in store_groups_of(ctiles):
                    yout = y[
                        row0 + sbase * ROWS_PER_TILE :
                        row0 + (sbase + scnt) * ROWS_PER_TILE,
                        :,
                    ].rearrange("(p r) c -> p (r c)", p=P)
                    nc.sync.dma_start(
                        yout,
                        out_t[:, sbase * TILE_COLS : (sbase + scnt) * TILE_COLS],
                    )
    nc.compile()
    return nc


def _make_consts(M: np.ndarray, c: np.ndarray, cast_in_dma: bool = True):
    tdt_np = ml_dtypes.bfloat16 if cast_in_dma else np.float32
    ident = np.eye(P, dtype=tdt_np)
    rmat = np.zeros((P, TILE_COLS), dtype=ml_dtypes.bfloat16)
    # R[9k+j, 9k+i] = M[i, j]  ->  block-diagonal of M^T
    rmat[:TILE_COLS, :] = np.kron(np.eye(G, dtype=np.float64), M.T).astype(
        ml_dtypes.bfloat16
    )
    # bias via the two all-ones lhsT rows: c = c_hi + c_lo (bf16 hi/lo split)
    c_hi = c.astype(ml_dtypes.bfloat16)
    c_lo = (c - c_hi.astype(np.float64)).astype(ml_dtypes.bfloat16)
    rmat[TILE_COLS, :] = np.tile(c_hi, G)
    rmat[TILE_COLS + 1, :] = np.tile(c_lo, G)
    return {"ident": ident, "rmat": rmat}


_NC_CACHE: dict = {}


def _cfg_key(cfg):
    return tuple(sorted((k, tuple(v) if isinstance(v, (list, tuple)) else v)
                        for k, v in cfg.items()))


def _get_nc(cfg):
    key = _cfg_key(cfg)
    if key not in _NC_CACHE:
        _NC_CACHE[key] = _build_nc(cfg)
    return _NC_CACHE[key]


def run_with_cfg(x, weights, biases, cfg, trace=False):
    M, c = _affine(np.asarray(weights), np.asarray(biases))
    nc = _get_nc(cfg)
    consts = _make_consts(M, c, cfg["cast_in_dma"])

    n = x.shape[0]
    # Overlapping shards: core i covers rows [s_i, s_i + ROWS_PC)
    starts = [(n - ROWS_PC) * i // (N_CORES - 1) for i in range(N_CORES)]
    in_maps = []
    for s in starts:
        in_maps.append(
            {
                "x": np.ascontiguousarray(x[s : s + ROWS_PC]),
                **consts,
            }
        )

    res = run_bass_kernel_spmd(
        nc, in_maps, core_ids=list(range(N_CORES)), trace=trace
    )
    out = np.empty((n, 9), dtype=np.float32)
    for s, r in zip(starts, res.results):
        out[s : s + ROWS_PC] = r["y"]
    return out, res


def kernel(input: np.ndarray, weights: np.ndarray, biases: np.ndarray) -> np.ndarray:
    x = np.ascontiguousarray(np.asarray(input, dtype=np.float32))
    assert x.shape == (N_TOTAL, 9), f"unexpected input shape {x.shape}"

    trace = os.environ.get("NNCONV_TRACE", "0") == "1"
    out, res = run_with_cfg(x, weights, biases, dict(DEFAULT_CFG), trace=trace)
    global _LAST_RESULTS
    _LAST_RESULTS = res
    if trace and res.exec_time_ns is not None:
        print(f"HW exec time: {res.exec_time_ns} ns")
        if res.instructions_and_trace is not None:
            print(f"trace: {res.instructions_and_trace[1]}")
    return out
